# revision 41
# baseline (speedup 1.0000x reference)
"""CodeSage attention (B=2, S=2048, H=1024, 16 heads x 64) on 8 Trainium2 cores.

Sharding: (batch x head-group) — core c handles batch c//4 and heads
[4*(c%4), 4*(c%4)+4) as two head-PAIRS. Versus head-only sharding this
halves the per-core hidden_states DMA (4MB, the HBM-bound phase-1 pacer),
halves the output partial (2048x1024), and frees 32KB/partition of SBUF
for the probs lookahead bank. The host sums the 4 partials per batch and
adds c_proj_b + bv @ c_proj_w (the V-bias reduces to a constant row).

Device-side design (bf16 matmuls, fp32 accumulation), per core:

phase 1: qT,kT [128=2*64, pair, 2048] = Wslice^T @ hsT  (1/sqrt(hd) folded
    into wq host-side). V natural per key tile ([128 tok, 256] = both
    pairs); the PSUM->aug copy fuses the exp(mask) row scale:
    aug[g2 = tile*2 + pair] = [ v'_ha | em64 | v'_hb ],  v' = v*em.

phase 2, NIT=8 iterations i = (query-block qb)*2 + pair p over 512-query
blocks: scoresT windows [128 keys, 1024 = 512q x 2 heads] row-tiled on the
PE (K=64 pairs run concurrently), exp on ScalarE (or the i16-Schraudolph
DVE fast path for FAST_SET tiles of iters >= 2), aug matmuls accumulate
[ctx|sumexp], normalize via approx-reciprocal after a DMA partition
realign, and c_proj accumulates BOTH pairs of a query block into one PSUM
bank pair (start/stop accumulation) before a single bf16 evacuation + DMA.

Score/exp windows are metered into the PE stream (~1 per 1.1us of PE work,
2-window batches) because each window's matmul waits on exp(w-2) through
the double-buffered score PSUM, and a waiting matmul blocks the PE FIFO.
Phase 1 banks up to CAP pre-computed windows which absorbs the ILVs' exp
deficit. A dummy-matmul warmup trips the HAM clock gate during the ramp.
"""

import numpy as np
import ml_dtypes
from collections import deque

B, S, H = 2, 2048, 1024
NH, HD = 16, 64
NCORES = 8
HPC = 4                     # heads per core
NP = 2                      # head pairs per core
DC = HPC * HD               # per-core head dims = 256
TC = S                      # per-core tokens = 2048 (one batch)
KC = H // 128               # 8 contraction chunks
SQB = TC // 512             # 4 query blocks
SKT = TC // 128             # 16 key tiles
NIT = SQB * NP              # 8 iterations (qb major, pair minor)
NG2 = SKT * NP              # 32 aug slabs (tile*2 + pair)

_CACHE = {}


def _build_nc():
    import concourse.mybir as mybir
    import concourse.tile as tile
    from concourse import bacc

    f32 = mybir.dt.float32
    bf16 = mybir.dt.bfloat16
    i16 = mybir.dt.int16

    nc = bacc.Bacc("TRN2", target_bir_lowering=False, debug=False,
                   num_devices=NCORES)

    hsT_d = nc.dram_tensor("hsT", [H, TC], bf16, kind="ExternalInput")
    wq_d = nc.dram_tensor("wq", [128, KC, DC], bf16, kind="ExternalInput")
    wk_d = nc.dram_tensor("wk", [128, KC, DC], bf16, kind="ExternalInput")
    wv_d = nc.dram_tensor("wv", [128, KC, DC], bf16, kind="ExternalInput")
    wp_d = nc.dram_tensor("wp", [128, NP, H], bf16, kind="ExternalInput")
    bq_d = nc.dram_tensor("bq", [128, NP], f32, kind="ExternalInput")
    bk_d = nc.dram_tensor("bk", [128, NP], f32, kind="ExternalInput")
    # mask duplicated per pair host-side: [128 key-in-tile, g2 = tile*2+pair]
    mask_d = nc.dram_tensor("mask", [128, NG2], f32, kind="ExternalInput")
    out_d = nc.dram_tensor("out", [TC, H], bf16, kind="ExternalOutput")

    EXP = mybir.ActivationFunctionType.Exp
    MULT = mybir.AluOpType.mult
    ADDOP = mybir.AluOpType.add
    # 1-term bf16 Schraudolph exp: pr_bf16 = bitcast_i16(round(s*A16 + B16)).
    # bf16 is the top 16 bits of f32, so the rounded i16 IS the bf16 prob —
    # one DVE tensor_scalar per tile. Sawtooth rel err rms 1.8%, mean-free;
    # any common scale cancels exactly in the softmax normalization.
    SCH_A16 = 12102203.161561485 / 65536.0   # 2^7 * log2(e)
    SCH_B16 = 16248.576                      # 2^7 * (127 - 0.058)
    FAST_SET = (2, 6, 10, 14)           # tile indices on the DVE fast path

    with tile.TileContext(nc) as tc:
        with (
            tc.tile_pool(name="const", bufs=1) as cpool,
            tc.tile_pool(name="qkv", bufs=1) as qpool,
            tc.tile_pool(name="probs", bufs=55) as ppool,
            tc.tile_pool(name="ctxn", bufs=3) as npool,
            tc.tile_pool(name="rcin", bufs=2) as ripool,
            tc.tile_pool(name="rec", bufs=2) as rpool,
            tc.tile_pool(name="ob", bufs=3) as opool,
            tc.tile_pool(name="ps_sc", bufs=2, space="PSUM") as pssc,
        ):
            wq_sb = cpool.tile([128, KC, DC], bf16)
            wk_sb = cpool.tile([128, KC, DC], bf16)
            wv_sb = cpool.tile([128, KC, DC], bf16)
            wp_sb = cpool.tile([128, NP, H], bf16)
            bq_sb = cpool.tile([128, NP], f32)
            bk_sb = cpool.tile([128, NP], f32)
            ones64 = cpool.tile([128, 64], bf16)
            mask_sb = cpool.tile([128, NG2], f32)
            em_sb = cpool.tile([128, NG2], f32)

            # ramp-critical DMAs only: wk+wq k0-3 halves gate the first proj
            # wave; k4-7 follow the hs quarter-0 chunks.
            nc.sync.dma_start(wk_sb[:, 0:4, :], wk_d.ap()[:, 0:4, :])
            nc.sync.dma_start(wq_sb[:, 0:4, :], wq_d.ap()[:, 0:4, :])
            nc.vector.memset(ones64[:], 1.0)

            qT_sb = qpool.tile([128, NP, TC], bf16)  # rows 0:64 / 64:128 = heads of pair
            kT_sb = qpool.tile([128, NP, TC], bf16)
            # aug stationaries: per slab g2, [v'_ha | em64 | v'_hb]
            aug_sb = qpool.tile([128, NG2, 192], bf16)

            # ---- phase-2 emission helpers ---------------------------------
            def emit_sc(i, skt, fast=False):
                qb, p = divmod(i, NP)
                sq = slice(qb * 512, qb * 512 + 512)
                sk = slice(skt * 128, skt * 128 + 128)
                sc_ps = pssc.tile([128, 1024], f32, tag="sc", name="sc_ps")
                nc.tensor.matmul(sc_ps[:, 0:512], lhsT=kT_sb[0:64, p, sk],
                                 rhs=qT_sb[0:64, p, sq], start=True, stop=True,
                                 skip_group_check=True)
                nc.tensor.matmul(sc_ps[:, 512:1024], lhsT=kT_sb[64:128, p, sk],
                                 rhs=qT_sb[64:128, p, sq], start=True, stop=True,
                                 skip_group_check=True)
                pr = ppool.tile([128, 1024], bf16, tag="pr", name="pr")
                if fast:
                    nc.vector.tensor_scalar(pr[:].bitcast(i16), sc_ps[:],
                                            SCH_A16, SCH_B16,
                                            op0=MULT, op1=ADDOP)
                else:
                    nc.scalar.activation(pr[:], sc_ps[:], EXP)
                return pr

            # ---- phase 1: QKV projection ----------------------------------
            with (
                tc.tile_pool(name="hs", bufs=1) as hpool,
                tc.tile_pool(name="ps1", bufs=1, space="PSUM") as ps1,
            ):
                hs_all = hpool.tile([128, KC, TC], bf16)
                # PE warm-up: the HAM clock gate holds PE at 1.2 GHz until
                # ~3.4us of sustained activity; the DMA ramp leaves PE idle
                # anyway, so dependency-free dummy matmuls trip the gate.
                warm_ps = pssc.tile([128, 1024], f32, tag="sc", name="sc_ps")
                for _ in range(130):
                    nc.tensor.matmul(warm_ps[0:64, 0:64], lhsT=ones64[:],
                                     rhs=ones64[:], start=True, stop=True,
                                     skip_group_check=True)
                # quarter-chunk transfers in wave-need order: wave 1 reads
                # quarter 0, wave 2 reads quarter 2 (kT p0 blocks 2-3),
                # quarters 1 and 3 are only needed from wave 4 on.
                for qi, q4 in enumerate((0, 2, 1, 3)):
                    cs = slice(q4 * 512, (q4 + 1) * 512)
                    for k in range(KC):
                        nc.sync.dma_start(hs_all[:, k, cs],
                                          hsT_d.ap()[k * 128:(k + 1) * 128, cs])
                    if qi == 0:
                        nc.sync.dma_start(wk_sb[:, 4:8, :], wk_d.ap()[:, 4:8, :])
                        nc.sync.dma_start(wq_sb[:, 4:8, :], wq_d.ap()[:, 4:8, :])
                        nc.sync.dma_start(bq_sb[:], bq_d.ap())
                        nc.sync.dma_start(bk_sb[:], bk_d.ap())
                        nc.sync.dma_start(mask_sb[:], mask_d.ap())
                        nc.scalar.activation(em_sb[:], mask_sb[:], EXP)
                    elif qi == 1:
                        nc.sync.dma_start(wv_sb[:], wv_d.ap())
                    elif qi == 2:
                        nc.sync.dma_start(wp_sb[:], wp_d.ap())

                # ---- globally smoothed score/exp emission -----------------
                CAP = 37                  # ppool(55) - 16 in-flight - margin
                probs_q = deque()
                cur = [0, 0]
                sched = {"credit": 2200.0, "allowed": 0}

                def _can_emit():
                    return (cur[0] < NIT and len(probs_q) < CAP
                            and cur[0] * SKT + cur[1] < sched["allowed"])

                def emit_one():
                    i, j = cur
                    fast = j in FAST_SET and i >= 2
                    probs_q.append(emit_sc(i, j, fast))
                    cur[:] = (i, j + 1) if j + 1 < SKT else (i + 1, 0)

                def tick(ns):
                    sched["credit"] += ns
                    while sched["credit"] >= 2200 and _can_emit():
                        emit_one()
                        if _can_emit():
                            emit_one()
                        sched["credit"] -= 2200
                    sched["credit"] = min(sched["credit"], 6600.0)

                def force_fill(n):
                    while len(probs_q) < n and cur[0] < NIT:
                        emit_one()

                def proj_joint(lanes):
                    """k-major accumulation over (w_off, dst_pair, blk, bias)
                    lanes — paces with arriving hs chunks."""
                    ps = [ps1.tile([128, 512], f32, tag=f"b{j}", name=f"ps_j{j}")
                          for j in range(len(lanes))]
                    for k in range(KC):
                        for j, (w_sb, dst_sb, bias_sb, p, blk) in enumerate(lanes):
                            nc.tensor.matmul(
                                ps[j][:], lhsT=w_sb[:, k, p * 128:(p + 1) * 128],
                                rhs=hs_all[:, k, blk * 512:(blk + 1) * 512],
                                start=(k == 0), stop=(k == KC - 1),
                                skip_group_check=True)
                        tick(213 * len(lanes))
                    for j, (w_sb, dst_sb, bias_sb, p, blk) in enumerate(lanes):
                        cols = slice(blk * 512, (blk + 1) * 512)
                        nc.vector.tensor_scalar_add(dst_sb[:, p, cols], ps[j][:],
                                                    bias_sb[:, p:p + 1])

                def KL(p, blk):
                    return (wk_sb, kT_sb, bk_sb, p, blk)

                def QL(p, blk):
                    return (wq_sb, qT_sb, bq_sb, p, blk)

                # pair-0 K plus its first queries first; gates open per wave.
                proj_joint([KL(0, 0), KL(0, 1), QL(0, 0)])
                sched["allowed"] = 8           # iter 0, keys 0:1024
                proj_joint([KL(0, 2), KL(0, 3), KL(1, 0), KL(1, 1)])
                sched["allowed"] = 16          # iter 0 complete
                proj_joint([QL(1, 0), KL(1, 2), KL(1, 3)])
                sched["allowed"] = 32          # iter 1 (kT p1 + qT p1 b0)
                proj_joint([QL(0, 1), QL(1, 1)])
                sched["allowed"] = 64          # iters 2-3
                proj_joint([QL(0, 2), QL(0, 3), QL(1, 2), QL(1, 3)])
                sched["allowed"] = NIT * SKT   # everything

                # V natural per key tile (both pairs at once, N=256); the
                # PSUM->aug copies fuse the em scaling.
                em_bc = em_sb[:].unsqueeze(2)
                nc.vector.tensor_scalar_mul(
                    aug_sb[:, :, 64:128], em_bc.broadcast_to([128, NG2, 64]),
                    1.0)
                for t in range(SKT):
                    gc = slice(t * 128, (t + 1) * 128)
                    v_ps = ps1.tile([128, DC], f32, tag=f"b{t % 4}", name="v_ps")
                    for k in range(KC):
                        nc.tensor.matmul(v_ps[:], lhsT=hs_all[:, k, gc],
                                         rhs=wv_sb[:, k, :],
                                         start=(k == 0), stop=(k == KC - 1),
                                         skip_group_check=True)
                    for p in range(NP):
                        g2 = t * NP + p
                        em = em_sb[:, g2:g2 + 1]
                        aug_v = aug_sb[:, g2, :].rearrange(
                            "p (a b) -> p a b", a=3)[:, 0:3:2, :]
                        src_v = v_ps[:, p * 128:(p + 1) * 128].rearrange(
                            "p (two c) -> p two c", two=2)
                        nc.vector.tensor_scalar_mul(aug_v, src_v, em)
                    tick(880)

            # ---- phase 2: attention + c_proj ------------------------------
            with tc.tile_pool(name="ps_ab", bufs=2, space="PSUM") as psab:
                # prev = [ctxn_p0, ctxn_p1, sq0, next_chunk] pending c_proj
                prev = None
                ctxn_hold = None

                def emit_cproj_chunk(c0, c1, sq0, t4, final=False):
                    # one 128-token chunk: both pairs accumulate into one
                    # PSUM bank pair before a single bf16 evacuation + DMA
                    # (output DMA rides the idle GpSimd queue).
                    tok = slice(t4 * 128, (t4 + 1) * 128)
                    rows = slice(sq0 + t4 * 128, sq0 + (t4 + 1) * 128)
                    if final and t4 % 2 == 1:
                        # after the last normalize psA/psB are dead; bank
                        # alternation lets consecutive chunks' matmuls run
                        # without waiting on each other's CAST
                        op_a = psab.tile([128, 512], f32, tag="pa", name="psA")
                        op_b = psab.tile([128, 512], f32, tag="pb", name="psB")
                    else:
                        op_a = psab.tile([128, 512], f32, tag="pa", name="op_a")
                        op_b = psab.tile([128, 512], f32, tag="pb", name="op_b")
                    for ci, ctxn in ((0, c0), (1, c1)):
                        st, sp = (ci == 0), (ci == 1)
                        nc.tensor.matmul(op_a[:], lhsT=ctxn[:, tok],
                                         rhs=wp_sb[:, ci, 0:512],
                                         start=st, stop=sp, skip_group_check=True)
                        nc.tensor.matmul(op_b[:], lhsT=ctxn[:, tok],
                                         rhs=wp_sb[:, ci, 512:1024],
                                         start=st, stop=sp, skip_group_check=True)
                    ob = opool.tile([128, 1024], bf16, tag="ob", name="ob")
                    nc.vector.tensor_copy(ob[:, 0:512], op_a[:])
                    if final:
                        nc.scalar.copy(ob[:, 512:1024], op_b[:])
                    else:
                        nc.vector.tensor_copy(ob[:, 512:1024], op_b[:])
                    nc.gpsimd.dma_start(out_d.ap()[rows, :], ob[:])
                    tick(800)

                force_fill(SKT)
                probs = [probs_q.popleft() for _ in range(SKT)]
                for i in range(NIT):
                    qb, p = divmod(i, NP)
                    sq0 = qb * 512
                    # --- ILV: aug matmuls of i, metered lookahead score/exp
                    # emission, and pending c_proj chunks spread through ---
                    psA = psab.tile([128, 512], f32, tag="pa", name="psA")
                    psB = psab.tile([128, 512], f32, tag="pb", name="psB")
                    for skt in range(SKT):
                        g2 = skt * NP + p
                        st, sp = (skt == 0), (skt == SKT - 1)
                        nc.tensor.matmul(psA[:], lhsT=aug_sb[:, g2, 0:128],
                                         rhs=probs[skt][:, 0:512], start=st, stop=sp,
                                         skip_group_check=True)
                        nc.tensor.matmul(psB[:], lhsT=aug_sb[:, g2, 64:192],
                                         rhs=probs[skt][:, 512:1024], start=st, stop=sp,
                                         skip_group_check=True)
                        tick(800)
                        if prev is not None and skt in (4, 10):
                            emit_cproj_chunk(prev[0], prev[1], prev[2], prev[3])
                            prev[3] += 1
                            if prev[3] == 4:
                                prev = None
                    # --- N: normalize --------------------------------------
                    se_st = ripool.tile([128, 512], f32, tag="st", name="se_st")
                    nc.vector.tensor_copy(se_st[64:128, :], psA[64:128, :])
                    nc.vector.tensor_copy(se_st[0:64, :], psB[0:64, :])
                    rec_in = ripool.tile([128, 512], f32, tag="ri", name="rec_in")
                    nc.sync.dma_start(rec_in[0:64, :], se_st[64:128, :])
                    nc.sync.dma_start(rec_in[64:128, :], se_st[0:64, :])
                    rec = rpool.tile([128, 512], f32, tag="rc", name="rec")
                    nc.vector.reciprocal_approx_fast(rec[:], rec_in[:])
                    ctxn = npool.tile([128, 512], bf16, tag="cn", name="ctxn")
                    nc.vector.tensor_tensor(ctxn[0:64, :], psA[0:64, :],
                                            rec[0:64, :], op=MULT)
                    nc.vector.tensor_tensor(ctxn[64:128, :], psB[64:128, :],
                                            rec[64:128, :], op=MULT)
                    if p == 0:
                        ctxn_hold = ctxn
                    else:
                        prev = [ctxn_hold, ctxn, sq0, 0]
                    if i + 1 < NIT:
                        force_fill(SKT)
                        probs = [probs_q.popleft() for _ in range(SKT)]

                for t4 in range(prev[3], 4):
                    emit_cproj_chunk(prev[0], prev[1], prev[2], t4, final=True)

    nc.compile()
    return nc


def _get_nc():
    if "nc" not in _CACHE:
        _CACHE["nc"] = _build_nc()
    return _CACHE["nc"]


def kernel(hidden_states, attention_mask, c_attn_w, c_attn_b, c_proj_w, c_proj_b):
    from concourse.bass_utils import run_bass_kernel_spmd

    bf16 = ml_dtypes.bfloat16
    hs = np.asarray(hidden_states, dtype=np.float32)          # [B, S, H]
    mask_full = np.broadcast_to(
        np.asarray(attention_mask, dtype=np.float32).reshape(B, 1, 1, S)[:, 0, 0, :],
        (B, S))
    w = np.asarray(c_attn_w, dtype=np.float32)
    bqkv = np.asarray(c_attn_b, dtype=np.float32)
    wp_full = np.asarray(c_proj_w, dtype=np.float32)
    scale = 1.0 / np.sqrt(HD)

    def pack(a):  # [H, DC] -> [128, KC, DC], contiguous per-partition lines
        return np.ascontiguousarray(
            a.reshape(KC, 128, DC).transpose(1, 0, 2)).astype(bf16)

    in_maps = []
    for c in range(NCORES):
        b = c // 4
        lo = (c % 4) * DC
        hi = lo + DC
        hsT = np.ascontiguousarray(hs[b].T).astype(bf16)      # [H, TC]
        # mask per key tile, duplicated per pair: [128, NG2]
        m = mask_full[b].reshape(SKT, 128)
        m2 = np.repeat(m[:, None, :], NP, axis=1).reshape(NG2, 128)
        in_maps.append({
            "hsT": hsT,
            "wq": pack(w[:, lo:hi] * scale),
            "wk": pack(w[:, H + lo:H + hi]),
            "wv": pack(w[:, 2 * H + lo:2 * H + hi]),
            "wp": np.ascontiguousarray(
                wp_full[lo:hi, :].reshape(NP, 128, H).transpose(1, 0, 2)
            ).astype(bf16),
            "bq": np.ascontiguousarray(
                (bqkv[lo:hi] * scale).reshape(NP, 128).T),
            "bk": np.ascontiguousarray(
                bqkv[H + lo:H + hi].reshape(NP, 128).T),
            "mask": np.ascontiguousarray(m2.T),
        })

    res = run_bass_kernel_spmd(_get_nc(), in_maps, core_ids=list(range(NCORES)))
    _CACHE["last_result"] = res
    acc = np.zeros((B, S, H), dtype=np.float32)
    for c in range(NCORES):
        acc[c // 4] += np.asarray(res.results[c]["out"], dtype=np.float32)
    # v-bias contributes the constant row bv @ c_proj_w (exact, host-side)
    bv_full = bqkv[2 * H:3 * H]
    acc += (bv_full @ wp_full + np.asarray(c_proj_b, dtype=np.float32))[None, None, :]
    return acc


# revision 44
# speedup vs baseline: 1.0183x; 1.0183x over previous
"""CodeSage attention (B=2, S=2048, H=1024, 16 heads x 64) on 8 Trainium2 cores.

Sharding: (batch x head-group) — core c handles batch c//4 and heads
[4*(c%4), 4*(c%4)+4) as two head-PAIRS. Versus head-only sharding this
halves the per-core hidden_states DMA (4MB, the HBM-bound phase-1 pacer),
halves the output partial (2048x1024), and frees 32KB/partition of SBUF
for the probs lookahead bank. The host sums the 4 partials per batch and
adds c_proj_b + bv @ c_proj_w (the V-bias reduces to a constant row).

Device-side design (bf16 matmuls, fp32 accumulation), per core:

phase 1: qT,kT [128=2*64, pair, 2048] = Wslice^T @ hsT  (1/sqrt(hd) folded
    into wq host-side). V natural per key tile ([128 tok, 256] = both
    pairs); the PSUM->aug copy fuses the exp(mask) row scale:
    aug[g2 = tile*2 + pair] = [ v'_ha | em64 | v'_hb ],  v' = v*em.

phase 2, NIT=8 iterations i = (query-block qb)*2 + pair p over 512-query
blocks: scoresT windows [128 keys, 1024 = 512q x 2 heads] row-tiled on the
PE (K=64 pairs run concurrently), exp on ScalarE (or the i16-Schraudolph
DVE fast path for FAST_SET tiles of iters >= 2), aug matmuls accumulate
[ctx|sumexp], normalize via approx-reciprocal after a DMA partition
realign, and c_proj accumulates BOTH pairs of a query block into one PSUM
bank pair (start/stop accumulation) before a single bf16 evacuation + DMA.

Score/exp windows are metered into the PE stream (~1 per 1.1us of PE work,
2-window batches) because each window's matmul waits on exp(w-2) through
the double-buffered score PSUM, and a waiting matmul blocks the PE FIFO.
Phase 1 banks up to CAP pre-computed windows which absorbs the ILVs' exp
deficit. A dummy-matmul warmup trips the HAM clock gate during the ramp.
"""

import numpy as np
import ml_dtypes
from collections import deque

B, S, H = 2, 2048, 1024
NH, HD = 16, 64
NCORES = 8
HPC = 4                     # heads per core
NP = 2                      # head pairs per core
DC = HPC * HD               # per-core head dims = 256
TC = S                      # per-core tokens = 2048 (one batch)
KC = H // 128               # 8 contraction chunks
SQB = TC // 512             # 4 query blocks
SKT = TC // 128             # 16 key tiles
NIT = SQB * NP              # 8 iterations (qb major, pair minor)
NG2 = SKT * NP              # 32 aug slabs (tile*2 + pair)

_CACHE = {}


def _build_nc():
    import concourse.mybir as mybir
    import concourse.tile as tile
    from concourse import bacc

    f32 = mybir.dt.float32
    bf16 = mybir.dt.bfloat16
    i16 = mybir.dt.int16

    nc = bacc.Bacc("TRN2", target_bir_lowering=False, debug=False,
                   num_devices=NCORES)

    hsT_d = nc.dram_tensor("hsT", [H, TC], bf16, kind="ExternalInput")
    wq_d = nc.dram_tensor("wq", [128, KC, DC], bf16, kind="ExternalInput")
    wk_d = nc.dram_tensor("wk", [128, KC, DC], bf16, kind="ExternalInput")
    wv_d = nc.dram_tensor("wv", [128, KC, DC], bf16, kind="ExternalInput")
    wp_d = nc.dram_tensor("wp", [128, NP, H], bf16, kind="ExternalInput")
    bq_d = nc.dram_tensor("bq", [128, NP], f32, kind="ExternalInput")
    bk_d = nc.dram_tensor("bk", [128, NP], f32, kind="ExternalInput")
    # mask duplicated per pair host-side: [128 key-in-tile, g2 = tile*2+pair]
    mask_d = nc.dram_tensor("mask", [128, NG2], f32, kind="ExternalInput")
    out_d = nc.dram_tensor("out", [TC, H], bf16, kind="ExternalOutput")

    EXP = mybir.ActivationFunctionType.Exp
    MULT = mybir.AluOpType.mult
    ADDOP = mybir.AluOpType.add
    # 1-term bf16 Schraudolph exp: pr_bf16 = bitcast_i16(round(s*A16 + B16)).
    # bf16 is the top 16 bits of f32, so the rounded i16 IS the bf16 prob —
    # one DVE tensor_scalar per tile. Sawtooth rel err rms 1.8%, mean-free;
    # any common scale cancels exactly in the softmax normalization.
    SCH_A16 = 12102203.161561485 / 65536.0   # 2^7 * log2(e)
    SCH_B16 = 16248.576                      # 2^7 * (127 - 0.058)
    FAST_SET = (2, 6, 10, 14)           # tile indices on the DVE fast path

    with tile.TileContext(nc) as tc:
        with (
            tc.tile_pool(name="const", bufs=1) as cpool,
            tc.tile_pool(name="qkv", bufs=1) as qpool,
            tc.tile_pool(name="probs", bufs=55) as ppool,
            tc.tile_pool(name="ctxn", bufs=3) as npool,
            tc.tile_pool(name="rcin", bufs=2) as ripool,
            tc.tile_pool(name="rec", bufs=2) as rpool,
            tc.tile_pool(name="ob", bufs=3) as opool,
            tc.tile_pool(name="ps_sc", bufs=2, space="PSUM") as pssc,
        ):
            wq_sb = cpool.tile([128, KC, DC], bf16)
            wk_sb = cpool.tile([128, KC, DC], bf16)
            wv_sb = cpool.tile([128, KC, DC], bf16)
            wp_sb = cpool.tile([128, NP, H], bf16)
            bq_sb = cpool.tile([128, NP], f32)
            bk_sb = cpool.tile([128, NP], f32)
            ones64 = cpool.tile([128, 64], bf16)
            mask_sb = cpool.tile([128, NG2], f32)
            em_sb = cpool.tile([128, NG2], f32)

            # ramp-critical DMAs only: wk+wq k0-3 halves gate the first proj
            # wave; k4-7 follow the hs quarter-0 chunks.
            nc.sync.dma_start(wk_sb[:, 0:4, :], wk_d.ap()[:, 0:4, :])
            nc.sync.dma_start(wq_sb[:, 0:4, :], wq_d.ap()[:, 0:4, :])
            nc.vector.memset(ones64[:], 1.0)

            qT_sb = qpool.tile([128, NP, TC], bf16)  # rows 0:64 / 64:128 = heads of pair
            kT_sb = qpool.tile([128, NP, TC], bf16)
            # aug stationaries: per slab g2, [v'_ha | em64 | v'_hb]
            aug_sb = qpool.tile([128, NG2, 192], bf16)

            # ---- phase-2 emission helpers ---------------------------------
            def emit_sc(i, skt, fast=False):
                qb, p = divmod(i, NP)
                sq = slice(qb * 512, qb * 512 + 512)
                sk = slice(skt * 128, skt * 128 + 128)
                sc_ps = pssc.tile([128, 1024], f32, tag="sc", name="sc_ps")
                nc.tensor.matmul(sc_ps[:, 0:512], lhsT=kT_sb[0:64, p, sk],
                                 rhs=qT_sb[0:64, p, sq], start=True, stop=True,
                                 skip_group_check=True)
                nc.tensor.matmul(sc_ps[:, 512:1024], lhsT=kT_sb[64:128, p, sk],
                                 rhs=qT_sb[64:128, p, sq], start=True, stop=True,
                                 skip_group_check=True)
                pr = ppool.tile([128, 1024], bf16, tag="pr", name="pr")
                if fast:
                    nc.vector.tensor_scalar(pr[:].bitcast(i16), sc_ps[:],
                                            SCH_A16, SCH_B16,
                                            op0=MULT, op1=ADDOP)
                else:
                    nc.scalar.activation(pr[:], sc_ps[:], EXP)
                return pr

            # ---- phase 1: QKV projection ----------------------------------
            with (
                tc.tile_pool(name="hs", bufs=1) as hpool,
                tc.tile_pool(name="ps1", bufs=1, space="PSUM") as ps1,
            ):
                hs_all = hpool.tile([128, KC, TC], bf16)
                # PE warm-up: the HAM clock gate holds PE at 1.2 GHz until
                # ~3.4us of sustained activity; the DMA ramp leaves PE idle
                # anyway, so dependency-free dummy matmuls trip the gate.
                warm_ps = pssc.tile([128, 1024], f32, tag="sc", name="sc_ps")
                for _ in range(90):
                    nc.tensor.matmul(warm_ps[0:64, 0:64], lhsT=ones64[:],
                                     rhs=ones64[:], start=True, stop=True,
                                     skip_group_check=True)
                # quarter-chunk transfers in wave-need order: wave 1 reads
                # quarter 0, wave 2 reads quarter 2 (kT p0 blocks 2-3),
                # quarters 1 and 3 are only needed from wave 4 on.
                for qi, q4 in enumerate((0, 2, 1, 3)):
                    cs = slice(q4 * 512, (q4 + 1) * 512)
                    for k in range(KC):
                        nc.sync.dma_start(hs_all[:, k, cs],
                                          hsT_d.ap()[k * 128:(k + 1) * 128, cs])
                    if qi == 0:
                        nc.sync.dma_start(wk_sb[:, 4:8, :], wk_d.ap()[:, 4:8, :])
                        nc.sync.dma_start(wq_sb[:, 4:8, :], wq_d.ap()[:, 4:8, :])
                        nc.sync.dma_start(bq_sb[:], bq_d.ap())
                        nc.sync.dma_start(bk_sb[:], bk_d.ap())
                        nc.sync.dma_start(mask_sb[:], mask_d.ap())
                        nc.scalar.activation(em_sb[:], mask_sb[:], EXP)
                    elif qi == 1:
                        nc.sync.dma_start(wv_sb[:], wv_d.ap())
                    elif qi == 2:
                        nc.sync.dma_start(wp_sb[:], wp_d.ap())

                # ---- globally smoothed score/exp emission -----------------
                CAP = 37                  # ppool(55) - 16 in-flight - margin
                probs_q = deque()
                cur = [0, 0]
                sched = {"credit": 2200.0, "allowed": 0}

                def _can_emit():
                    return (cur[0] < NIT and len(probs_q) < CAP
                            and cur[0] * SKT + cur[1] < sched["allowed"])

                def emit_one():
                    i, j = cur
                    fast = j in FAST_SET and i >= 2
                    probs_q.append(emit_sc(i, j, fast))
                    cur[:] = (i, j + 1) if j + 1 < SKT else (i + 1, 0)

                def tick(ns):
                    sched["credit"] += ns
                    while sched["credit"] >= 2200 and _can_emit():
                        emit_one()
                        if _can_emit():
                            emit_one()
                        sched["credit"] -= 2200
                    sched["credit"] = min(sched["credit"], 6600.0)

                def force_fill(n):
                    while len(probs_q) < n and cur[0] < NIT:
                        emit_one()

                def proj_joint(lanes):
                    """k-major accumulation over (w_off, dst_pair, blk, bias)
                    lanes — paces with arriving hs chunks."""
                    ps = [ps1.tile([128, 512], f32, tag=f"b{j}", name=f"ps_j{j}")
                          for j in range(len(lanes))]
                    for k in range(KC):
                        for j, (w_sb, dst_sb, bias_sb, p, blk) in enumerate(lanes):
                            nc.tensor.matmul(
                                ps[j][:], lhsT=w_sb[:, k, p * 128:(p + 1) * 128],
                                rhs=hs_all[:, k, blk * 512:(blk + 1) * 512],
                                start=(k == 0), stop=(k == KC - 1),
                                skip_group_check=True)
                        tick(213 * len(lanes))
                    for j, (w_sb, dst_sb, bias_sb, p, blk) in enumerate(lanes):
                        cols = slice(blk * 512, (blk + 1) * 512)
                        nc.vector.tensor_scalar_add(dst_sb[:, p, cols], ps[j][:],
                                                    bias_sb[:, p:p + 1])

                def KL(p, blk):
                    return (wk_sb, kT_sb, bk_sb, p, blk)

                def QL(p, blk):
                    return (wq_sb, qT_sb, bq_sb, p, blk)

                # pair-0 K plus its first queries first; gates open per wave.
                proj_joint([KL(0, 0), KL(0, 1), QL(0, 0)])
                sched["allowed"] = 8           # iter 0, keys 0:1024
                proj_joint([KL(0, 2), KL(0, 3), QL(1, 0)])
                sched["allowed"] = 16          # iter 0 complete
                proj_joint([KL(1, 0), KL(1, 1)])
                proj_joint([KL(1, 2), KL(1, 3), QL(0, 1), QL(1, 1)])
                sched["allowed"] = 64          # iters 1-3
                proj_joint([QL(0, 2), QL(0, 3), QL(1, 2), QL(1, 3)])
                sched["allowed"] = NIT * SKT   # everything

                # V natural per key tile (both pairs at once, N=256); the
                # PSUM->aug copies fuse the em scaling.
                em_bc = em_sb[:].unsqueeze(2)
                nc.vector.tensor_scalar_mul(
                    aug_sb[:, :, 64:128], em_bc.broadcast_to([128, NG2, 64]),
                    1.0)
                for t in range(SKT):
                    gc = slice(t * 128, (t + 1) * 128)
                    v_ps = ps1.tile([128, DC], f32, tag=f"b{t % 4}", name="v_ps")
                    for k in range(KC):
                        nc.tensor.matmul(v_ps[:], lhsT=hs_all[:, k, gc],
                                         rhs=wv_sb[:, k, :],
                                         start=(k == 0), stop=(k == KC - 1),
                                         skip_group_check=True)
                    for p in range(NP):
                        g2 = t * NP + p
                        em = em_sb[:, g2:g2 + 1]
                        aug_v = aug_sb[:, g2, :].rearrange(
                            "p (a b) -> p a b", a=3)[:, 0:3:2, :]
                        src_v = v_ps[:, p * 128:(p + 1) * 128].rearrange(
                            "p (two c) -> p two c", two=2)
                        nc.vector.tensor_scalar_mul(aug_v, src_v, em)
                    tick(880)

            # ---- phase 2: attention + c_proj ------------------------------
            with tc.tile_pool(name="ps_ab", bufs=2, space="PSUM") as psab:
                # prev = [ctxn_p0, ctxn_p1, sq0, next_chunk] pending c_proj
                prev = None
                ctxn_hold = None

                def emit_cproj_chunk(c0, c1, sq0, t4, final=False):
                    # one 128-token chunk: both pairs accumulate into one
                    # PSUM bank pair before a single bf16 evacuation + DMA
                    # (output DMA rides the idle GpSimd queue).
                    tok = slice(t4 * 128, (t4 + 1) * 128)
                    rows = slice(sq0 + t4 * 128, sq0 + (t4 + 1) * 128)
                    if final and t4 % 2 == 1:
                        # after the last normalize psA/psB are dead; bank
                        # alternation lets consecutive chunks' matmuls run
                        # without waiting on each other's CAST
                        op_a = psab.tile([128, 512], f32, tag="pa", name="psA")
                        op_b = psab.tile([128, 512], f32, tag="pb", name="psB")
                    else:
                        op_a = psab.tile([128, 512], f32, tag="pa", name="op_a")
                        op_b = psab.tile([128, 512], f32, tag="pb", name="op_b")
                    for ci, ctxn in ((0, c0), (1, c1)):
                        st, sp = (ci == 0), (ci == 1)
                        nc.tensor.matmul(op_a[:], lhsT=ctxn[:, tok],
                                         rhs=wp_sb[:, ci, 0:512],
                                         start=st, stop=sp, skip_group_check=True)
                        nc.tensor.matmul(op_b[:], lhsT=ctxn[:, tok],
                                         rhs=wp_sb[:, ci, 512:1024],
                                         start=st, stop=sp, skip_group_check=True)
                    ob = opool.tile([128, 1024], bf16, tag="ob", name="ob")
                    nc.vector.tensor_copy(ob[:, 0:512], op_a[:])
                    if final:
                        nc.scalar.copy(ob[:, 512:1024], op_b[:])
                    else:
                        nc.vector.tensor_copy(ob[:, 512:1024], op_b[:])
                    nc.gpsimd.dma_start(out_d.ap()[rows, :], ob[:])
                    tick(800)

                force_fill(SKT)
                probs = [probs_q.popleft() for _ in range(SKT)]
                for i in range(NIT):
                    qb, p = divmod(i, NP)
                    sq0 = qb * 512
                    # --- ILV: aug matmuls of i, metered lookahead score/exp
                    # emission, and pending c_proj chunks spread through ---
                    psA = psab.tile([128, 512], f32, tag="pa", name="psA")
                    psB = psab.tile([128, 512], f32, tag="pb", name="psB")
                    for skt in range(SKT):
                        g2 = skt * NP + p
                        st, sp = (skt == 0), (skt == SKT - 1)
                        nc.tensor.matmul(psA[:], lhsT=aug_sb[:, g2, 0:128],
                                         rhs=probs[skt][:, 0:512], start=st, stop=sp,
                                         skip_group_check=True)
                        nc.tensor.matmul(psB[:], lhsT=aug_sb[:, g2, 64:192],
                                         rhs=probs[skt][:, 512:1024], start=st, stop=sp,
                                         skip_group_check=True)
                        tick(800)
                        if prev is not None and skt in (4, 10):
                            emit_cproj_chunk(prev[0], prev[1], prev[2], prev[3])
                            prev[3] += 1
                            if prev[3] == 4:
                                prev = None
                    # --- N: normalize --------------------------------------
                    se_st = ripool.tile([128, 512], f32, tag="st", name="se_st")
                    nc.vector.tensor_copy(se_st[64:128, :], psA[64:128, :])
                    nc.vector.tensor_copy(se_st[0:64, :], psB[0:64, :])
                    rec_in = ripool.tile([128, 512], f32, tag="ri", name="rec_in")
                    nc.sync.dma_start(rec_in[0:64, :], se_st[64:128, :])
                    nc.sync.dma_start(rec_in[64:128, :], se_st[0:64, :])
                    rec = rpool.tile([128, 512], f32, tag="rc", name="rec")
                    nc.vector.reciprocal_approx_fast(rec[:], rec_in[:])
                    ctxn = npool.tile([128, 512], bf16, tag="cn", name="ctxn")
                    nc.vector.tensor_tensor(ctxn[0:64, :], psA[0:64, :],
                                            rec[0:64, :], op=MULT)
                    nc.vector.tensor_tensor(ctxn[64:128, :], psB[64:128, :],
                                            rec[64:128, :], op=MULT)
                    if p == 0:
                        ctxn_hold = ctxn
                    else:
                        prev = [ctxn_hold, ctxn, sq0, 0]
                    if i + 1 < NIT:
                        # top up past the bare 16 so the last iterations keep
                        # a lookahead cushion — otherwise the final windows
                        # retire at the very end and ILV(7) stalls on its
                        # own probs while ScalarE sits idle
                        force_fill(24)
                        probs = [probs_q.popleft() for _ in range(SKT)]

                for t4 in range(prev[3], 4):
                    emit_cproj_chunk(prev[0], prev[1], prev[2], t4, final=True)

    nc.compile()
    return nc


def _get_nc():
    if "nc" not in _CACHE:
        _CACHE["nc"] = _build_nc()
    return _CACHE["nc"]


def kernel(hidden_states, attention_mask, c_attn_w, c_attn_b, c_proj_w, c_proj_b):
    from concourse.bass_utils import run_bass_kernel_spmd

    bf16 = ml_dtypes.bfloat16
    hs = np.asarray(hidden_states, dtype=np.float32)          # [B, S, H]
    mask_full = np.broadcast_to(
        np.asarray(attention_mask, dtype=np.float32).reshape(B, 1, 1, S)[:, 0, 0, :],
        (B, S))
    w = np.asarray(c_attn_w, dtype=np.float32)
    bqkv = np.asarray(c_attn_b, dtype=np.float32)
    wp_full = np.asarray(c_proj_w, dtype=np.float32)
    scale = 1.0 / np.sqrt(HD)

    def pack(a):  # [H, DC] -> [128, KC, DC], contiguous per-partition lines
        return np.ascontiguousarray(
            a.reshape(KC, 128, DC).transpose(1, 0, 2)).astype(bf16)

    in_maps = []
    for c in range(NCORES):
        b = c // 4
        lo = (c % 4) * DC
        hi = lo + DC
        hsT = np.ascontiguousarray(hs[b].T).astype(bf16)      # [H, TC]
        # mask per key tile, duplicated per pair: [128, NG2]
        m = mask_full[b].reshape(SKT, 128)
        m2 = np.repeat(m[:, None, :], NP, axis=1).reshape(NG2, 128)
        in_maps.append({
            "hsT": hsT,
            "wq": pack(w[:, lo:hi] * scale),
            "wk": pack(w[:, H + lo:H + hi]),
            "wv": pack(w[:, 2 * H + lo:2 * H + hi]),
            "wp": np.ascontiguousarray(
                wp_full[lo:hi, :].reshape(NP, 128, H).transpose(1, 0, 2)
            ).astype(bf16),
            "bq": np.ascontiguousarray(
                (bqkv[lo:hi] * scale).reshape(NP, 128).T),
            "bk": np.ascontiguousarray(
                bqkv[H + lo:H + hi].reshape(NP, 128).T),
            "mask": np.ascontiguousarray(m2.T),
        })

    res = run_bass_kernel_spmd(_get_nc(), in_maps, core_ids=list(range(NCORES)))
    _CACHE["last_result"] = res
    acc = np.zeros((B, S, H), dtype=np.float32)
    for c in range(NCORES):
        acc[c // 4] += np.asarray(res.results[c]["out"], dtype=np.float32)
    # v-bias contributes the constant row bv @ c_proj_w (exact, host-side)
    bv_full = bqkv[2 * H:3 * H]
    acc += (bv_full @ wp_full + np.asarray(c_proj_b, dtype=np.float32))[None, None, :]
    return acc
